# revision 5
# baseline (speedup 1.0000x reference)
"""Trainium2 Bass kernel for nn_CBAMSpaceMask (CBAM spatial mask over T timestep blocks).

Math per timestep block t (3 channels):
  mx_c = maxpool3x3(x_c)          (stride 1, -inf pad == replicate pad)
  av_c = avgpool3x3(x_c)/9        (zero pad, count_include_pad)
  y_t  = sum_c wM_c * mx_c + wA_c * av_c + b   (3x3 conv, zero pad)
  out[3t+c] = sigmoid(leakyrelu(y_t))          (broadcast over c)

Device decomposition (per core = 1 batch image):
  - by-channel tiles [3*KB rows, 16 ts, 256 cols] bf16; input loaded from HBM
    ONCE; row-shifted U/D made by SBUF->SBUF DMA (+1-row HBM patches)
  - pools on DVE: vertical max via U/X/D, horizontal max + box-sum via col slices;
    avg vertical box folded into the conv operator (band-5 Toeplitz)
  - conv: channel-folded banded-Toeplitz matmuls on PE (bf16):
    K = 3 channels x KB pooled rows, M = y rows of chunk, N = 512 (2 ts packed);
    all 16 ts accumulate simultaneously across the 8 PSUM banks, operator-outer
  - epilogue: ACT Identity(+bias) -> DVE leaky -> ACT Sigmoid -> 6 DMA out/bank
Sharding: pure data parallel, batch dim across 8 cores.
"""
import sys

sys.path.insert(0, "/opt/trn_rl_repo")

import numpy as np
import ml_dtypes
from contextlib import ExitStack

import concourse.bass as bass
import concourse.tile as tile
from concourse import bacc, mybir
from concourse.bass_utils import run_bass_kernel_spmd

F32 = mybir.dt.float32
BF16 = mybir.dt.bfloat16

B, CTOT, H, W = 8, 48, 256, 256
T = 16
N_CORES = 8

# chunk geometry: y rows [m0, m1) from pooled rows [q0, q1); KB = q1 - q0
# q0 = max(0, m0-2) covers the band-5 avg operator; KB <= 42 so 3*KB <= 126.
CHUNKS = []
_m0 = 0
while _m0 < H:
    _m1 = min(_m0 + 38, H)
    _q0 = max(0, _m0 - 2)
    _q1 = min(_q0 + 42, H)
    CHUNKS.append((_m0, _m1, _q0, _q1))
    _m0 = _m1
NCH = len(CHUNKS)
NMAT = NCH * 6  # (chunk, path, kw)

_cache = {}


def _mat_index(ch, path, kw):
    return (ch * 2 + path) * 3 + kw


def _build_stack(conv_w):
    """lhsT stack [128, NMAT, 128] bf16.

    mats[ch][path][kw][k= c*KB + i, m] = OP_path_c[m0+m, q0+i], where
    OP_max_c  = sum_kh w[2c][kh,kw] eye(k=kh-1)            (band 3)
    OP_avg_c  = (sum_kh w[2c+1][kh,kw] eye(k=kh-1)) @ Bv/9 (band 5)
    """
    w = conv_w[0].astype(np.float64)  # [6, 3, 3]
    Bv = np.zeros((H, H))
    for i in (-1, 0, 1):
        Bv += np.eye(H, k=i)
    stack = np.zeros((128, NMAT, 128), dtype=np.float64)
    for ch, (m0, m1, q0, q1) in enumerate(CHUNKS):
        M = m1 - m0
        KB = q1 - q0
        for path in range(2):  # 0 = max (mx), 1 = avg (bh)
            for kw in range(3):
                mat = _mat_index(ch, path, kw)
                for c in range(3):
                    k2d = w[2 * c] if path == 0 else w[2 * c + 1]
                    op = np.zeros((H, H))
                    for kh in range(3):
                        op += k2d[kh, kw] * np.eye(H, k=kh - 1)
                    if path == 1:
                        op = (op @ Bv) / 9.0
                    stack[c * KB:(c + 1) * KB, mat, 0:M] = op[m0:m1, q0:q1].T
    return stack.astype(ml_dtypes.bfloat16)


def _build_program():
    nc = bacc.Bacc("TRN2", target_bir_lowering=False, debug=False, enable_asserts=False)
    x_ap = nc.dram_tensor("x", [CTOT, H, W], F32, kind="ExternalInput").ap()
    cst_ap = nc.dram_tensor("cst", [128, NMAT, 128], BF16, kind="ExternalInput").ap()
    bias_ap = nc.dram_tensor("bias", [128, 1], F32, kind="ExternalInput").ap()
    out_ap = nc.dram_tensor("out", [CTOT, H, W], F32, kind="ExternalOutput").ap()

    MAXOP = mybir.AluOpType.max
    ADDOP = mybir.AluOpType.add

    with tile.TileContext(nc) as tc, ExitStack() as ctx:
        const_pool = ctx.enter_context(tc.tile_pool(name="const", bufs=1))
        ld_pool = ctx.enter_context(tc.tile_pool(name="loads", bufs=2))
        tmp_pool = ctx.enter_context(tc.tile_pool(name="ptmp", bufs=2))
        mxbh_pool = ctx.enter_context(tc.tile_pool(name="mxbh", bufs=2))
        psum_pool = ctx.enter_context(tc.tile_pool(name="psum", bufs=1, space="PSUM"))
        epi_pool = ctx.enter_context(tc.tile_pool(name="epi", bufs=3))

        cst = const_pool.tile([128, NMAT, 128], BF16, tag="cst")
        nc.sync.dma_start(out=cst[:], in_=cst_ap)
        bias = const_pool.tile([128, 1], F32, tag="bias")
        nc.sync.dma_start(out=bias[:], in_=bias_ap)

        for ch, (m0, m1, q0, q1) in enumerate(CHUNKS):
            M = m1 - m0
            KB = q1 - q0
            P3 = 3 * KB

            # ---- loads: X once from HBM; U/D = SBUF shifts + 1-row patches
            X = ld_pool.tile([128, T, W], BF16, tag="ldx")
            U = ld_pool.tile([128, T, W], BF16, tag="ldu")
            D = ld_pool.tile([128, T, W], BF16, tag="ldd")
            for c in range(3):
                base = c * KB
                src = x_ap[c::3]  # [16, 256, 256] planes of channel c
                nc.gpsimd.dma_start(
                    out=X[base:base + KB],
                    in_=src[:, q0:q1, :].transpose([1, 0, 2]))
                # U[base+i] = x row q0+i+1
                nc.sync.dma_start(out=U[base:base + KB - 1],
                                  in_=X[base + 1:base + KB])
                if q1 < H:
                    nc.gpsimd.dma_start(
                        out=U[base + KB - 1:base + KB],
                        in_=src[:, q1:q1 + 1, :].transpose([1, 0, 2]))
                else:  # replicate bottom row (maxpool -inf pad)
                    nc.sync.dma_start(out=U[base + KB - 1:base + KB],
                                      in_=X[base + KB - 1:base + KB])
                # D[base+i] = x row q0+i-1
                nc.sync.dma_start(out=D[base + 1:base + KB],
                                  in_=X[base:base + KB - 1])
                if q0 > 0:
                    nc.gpsimd.dma_start(
                        out=D[base:base + 1],
                        in_=src[:, q0 - 1:q0, :].transpose([1, 0, 2]))
                else:  # replicate top row
                    nc.sync.dma_start(out=D[base:base + 1], in_=X[base:base + 1])

            # ---- pools: mx = 3x3 max (U/X/D vertical, slices horizontal);
            #             bh = horizontal box sum of X (zero pad).
            # mx/bh have a zero col on each side (data col j at tile col j+1)
            # so each kw tap is a full-width rhs slice with built-in zero pad.
            b_ = tmp_pool.tile([128, T, W], BF16, tag="poolb")
            vx = tmp_pool.tile([128, T, W], BF16, tag="poolvx")
            mx = mxbh_pool.tile([128, T, W + 2], BF16, tag="mx")
            bh = mxbh_pool.tile([128, T, W + 2], BF16, tag="bh")
            p = P3
            nc.vector.tensor_tensor(out=b_[:p], in0=U[:p], in1=D[:p], op=MAXOP)
            nc.vector.tensor_tensor(out=vx[:p], in0=b_[:p], in1=X[:p], op=MAXOP)
            for pad in (mx, bh):
                nc.vector.memset(pad[:p, :, 0:1], 0)
                nc.vector.memset(pad[:p, :, W + 1:W + 2], 0)
            nc.vector.tensor_tensor(out=mx[:p, :, 1:256], in0=vx[:p, :, 0:255],
                                    in1=vx[:p, :, 1:256], op=MAXOP)
            nc.vector.tensor_copy(mx[:p, :, 256:257], vx[:p, :, 255:256])
            nc.vector.tensor_tensor(out=mx[:p, :, 2:257], in0=mx[:p, :, 2:257],
                                    in1=vx[:p, :, 0:255], op=MAXOP)
            nc.vector.tensor_tensor(out=bh[:p, :, 1:256], in0=X[:p, :, 0:255],
                                    in1=X[:p, :, 1:256], op=ADDOP)
            nc.vector.tensor_copy(bh[:p, :, 256:257], X[:p, :, 255:256])
            nc.vector.tensor_tensor(out=bh[:p, :, 2:257], in0=bh[:p, :, 2:257],
                                    in1=X[:p, :, 0:255], op=ADDOP)

            # ---- matmuls: operator-outer; all 16 ts resident in 8 PSUM banks
            ps = [psum_pool.tile([128, 2, W], F32, tag=f"ps{b}", name=f"ps{b}")
                  for b in range(8)]
            ops = [(path, kw) for path in (0, 1) for kw in (1, 0, 2)]
            n = len(ops)
            for i, (path, kw) in enumerate(ops):
                src = mx if path == 0 else bh
                mat = _mat_index(ch, path, kw)
                lhsT = cst[0:P3, mat, 0:M]
                for b in range(8):
                    rhs = src[0:P3, 2 * b:2 * b + 2, kw:kw + W]
                    nc.tensor.matmul(ps[b][0:M], lhsT, rhs,
                                     start=(i == 0), stop=(i == n - 1))

            # ---- epilogue per bank: bias -> leaky -> sigmoid -> 6 DMAs
            for b in range(8):
                v = epi_pool.tile([128, 2, W], F32, tag="epiv")
                nc.scalar.activation(v[0:M], ps[b][0:M],
                                     mybir.ActivationFunctionType.Identity,
                                     bias=bias[0:M], scale=1.0)
                lk = epi_pool.tile([128, 2, W], F32, tag="epil")
                nc.vector.scalar_tensor_tensor(out=lk[0:M], in0=v[0:M], scalar=0.01,
                                               in1=v[0:M], op0=mybir.AluOpType.mult,
                                               op1=MAXOP)
                sg = epi_pool.tile([128, 2, W], F32, tag="epis")
                nc.scalar.activation(sg[0:M], lk[0:M],
                                     mybir.ActivationFunctionType.Sigmoid)
                for tt in range(2):
                    t = 2 * b + tt
                    for c in range(3):
                        nc.sync.dma_start(out=out_ap[3 * t + c, m0:m1, :],
                                          in_=sg[0:M, tt, :])

    nc.compile()
    return nc


def kernel(input_tensor, conv_w, conv_b):
    input_tensor = np.ascontiguousarray(np.asarray(input_tensor, dtype=np.float32))
    conv_w = np.asarray(conv_w, dtype=np.float32)
    conv_b = np.asarray(conv_b, dtype=np.float32)

    if "nc" not in _cache:
        _cache["nc"] = _build_program()
    nc = _cache["nc"]

    stack = _build_stack(conv_w)
    bias_vec = np.full((128, 1), conv_b[0], dtype=np.float32)
    in_maps = [
        {"x": input_tensor[i], "cst": stack, "bias": bias_vec}
        for i in range(N_CORES)
    ]
    res = run_bass_kernel_spmd(nc, in_maps, list(range(N_CORES)))
    out = np.stack([res.results[i]["out"] for i in range(N_CORES)], axis=0)
    return out.astype(np.float32)


if __name__ == "__main__":
    rng = np.random.default_rng(0)
    x = rng.standard_normal((B, CTOT, H, W), dtype=np.float32)
    cw = rng.uniform(-0.1, 0.1, (1, 6, 3, 3)).astype(np.float32)
    cb = np.array([0.01], dtype=np.float32)
    o = kernel(x, cw, cb)
    print(o.shape, o.dtype)


# revision 15
# speedup vs baseline: 1.1278x; 1.1278x over previous
"""Trainium2 Bass kernel for nn_CBAMSpaceMask (CBAM spatial mask over T timestep blocks).

Math per timestep block t (3 channels):
  mx_c = maxpool3x3(x_c)          (stride 1, -inf pad == replicate pad)
  av_c = avgpool3x3(x_c)/9        (zero pad, count_include_pad)
  y_t  = sum_c wM_c * mx_c + wA_c * av_c + b   (3x3 conv, zero pad)
  out[3t+c] = sigmoid(leakyrelu(y_t))          (broadcast over c)

Device decomposition (per core = 1 batch image):
  - by-channel tiles [3*KB rows, 16 ts, 256 cols] bf16; input loaded from HBM
    ONCE; row-shifted U/D made by SBUF->SBUF DMA (+1-row HBM patches)
  - pools on DVE: vertical max via U/X/D, horizontal max + box-sum via col slices;
    avg vertical box folded into the conv operator (band-5 Toeplitz)
  - conv: channel-folded banded-Toeplitz matmuls on PE (bf16):
    K = 3 channels x KB pooled rows, M = y rows of chunk, N = 512 (2 ts packed);
    all 16 ts accumulate simultaneously across the 8 PSUM banks, operator-outer
  - epilogue: ACT Identity(+bias) -> DVE leaky -> ACT Sigmoid -> 6 DMA out/bank
Sharding: pure data parallel, batch dim across 8 cores.
"""
import sys

sys.path.insert(0, "/opt/trn_rl_repo")

import numpy as np
import ml_dtypes
from contextlib import ExitStack

import concourse.bass as bass
import concourse.tile as tile
from concourse import bacc, mybir
from concourse.bass_utils import run_bass_kernel_spmd

F32 = mybir.dt.float32
BF16 = mybir.dt.bfloat16

B, CTOT, H, W = 8, 48, 256, 256
T = 16
N_CORES = 8

# chunk geometry: y rows [m0, m1) from pooled rows [q0, q1); KB = q1 - q0
# q0 = max(0, m0-2) covers the band-5 avg operator; KB <= 42 so 3*KB <= 126.
CHUNKS = []
_m0 = 0
while _m0 < H:
    _m1 = min(_m0 + 38, H)
    _q0 = max(0, _m0 - 2)
    _q1 = min(_q0 + 42, H)
    CHUNKS.append((_m0, _m1, _q0, _q1))
    _m0 = _m1
NCH = len(CHUNKS)
NMAT = NCH * 6  # (chunk, path, kw)

_cache = {}


def _mat_index(ch, path, kw):
    return (ch * 2 + path) * 3 + kw


def _build_stack(conv_w, conv_b):
    """lhsT stack [128, NMAT, 128] bf16.

    mats[ch][path][kw][k= c*KB + i, m] = OP_path_c[m0+m, q0+i], where
    OP_max_c  = sum_kh w[2c][kh,kw] eye(k=kh-1)            (band 3)
    OP_avg_c  = (sum_kh w[2c+1][kh,kw] eye(k=kh-1)) @ Bv/9 (band 5)
    """
    w = conv_w[0].astype(np.float64)  # [6, 3, 3]
    Bv = np.zeros((H, H))
    for i in (-1, 0, 1):
        Bv += np.eye(H, k=i)
    stack = np.zeros((128, NMAT, 128), dtype=np.float64)
    for ch, (m0, m1, q0, q1) in enumerate(CHUNKS):
        M = m1 - m0
        KB = q1 - q0
        for path in range(2):  # 0 = max (mx), 1 = avg (bh)
            for kw in range(3):
                mat = _mat_index(ch, path, kw)
                for c in range(3):
                    k2d = w[2 * c] if path == 0 else w[2 * c + 1]
                    op = np.zeros((H, H))
                    for kh in range(3):
                        op += k2d[kh, kw] * np.eye(H, k=kh - 1)
                    if path == 1:
                        op = (op @ Bv) / 9.0
                    stack[c * KB:(c + 1) * KB, mat, 0:M] = op[m0:m1, q0:q1].T
        # bias rides the first (start) matmul: ones-row at partition 3*KB of
        # the mx tile contracts with a bias-valued lhsT row.
        stack[3 * KB, _mat_index(ch, 0, 1), 0:M] = float(conv_b[0])
    return stack.astype(ml_dtypes.bfloat16)


def _build_program():
    nc = bacc.Bacc("TRN2", target_bir_lowering=False, debug=False, enable_asserts=False)
    x_ap = nc.dram_tensor("x", [CTOT, H, W], F32, kind="ExternalInput").ap()
    cst_ap = nc.dram_tensor("cst", [128, NMAT, 128], BF16, kind="ExternalInput").ap()
    out_ap = nc.dram_tensor("out", [CTOT, H, W], F32, kind="ExternalOutput").ap()

    MAXOP = mybir.AluOpType.max
    ADDOP = mybir.AluOpType.add

    with tile.TileContext(nc) as tc, ExitStack() as ctx:
        const_pool = ctx.enter_context(tc.tile_pool(name="const", bufs=1))
        ld_pool = ctx.enter_context(tc.tile_pool(name="loads", bufs=2))
        tmp_pool = ctx.enter_context(tc.tile_pool(name="ptmp", bufs=2))
        mxbh_pool = ctx.enter_context(tc.tile_pool(name="mxbh", bufs=2))
        psum_pool = ctx.enter_context(tc.tile_pool(name="psum", bufs=1, space="PSUM"))
        epi_pool = ctx.enter_context(tc.tile_pool(name="epi", bufs=3))

        cst = const_pool.tile([128, NMAT, 128], BF16, tag="cst")
        nc.sync.dma_start(out=cst[:], in_=cst_ap)
        ones = const_pool.tile([1, T, W + 2], BF16, tag="ones")
        nc.vector.memset(ones[:], 1.0)

        for ch, (m0, m1, q0, q1) in enumerate(CHUNKS):
            M = m1 - m0
            KB = q1 - q0
            P3 = 3 * KB

            # ---- loads: X once from HBM; U/D = SBUF shifts + 1-row patches
            X = ld_pool.tile([128, T, W], BF16, tag="ldx")
            U = ld_pool.tile([128, T, W], BF16, tag="ldu")
            D = ld_pool.tile([128, T, W], BF16, tag="ldd")
            for c in range(3):
                base = c * KB
                src = x_ap[c::3]  # [16, 256, 256] planes of channel c
                nc.gpsimd.dma_start(
                    out=X[base:base + KB],
                    in_=src[:, q0:q1, :].transpose([1, 0, 2]))
                # U[base+i] = x row q0+i+1
                nc.sync.dma_start(out=U[base:base + KB - 1],
                                  in_=X[base + 1:base + KB])
                if q1 < H:
                    nc.gpsimd.dma_start(
                        out=U[base + KB - 1:base + KB],
                        in_=src[:, q1:q1 + 1, :].transpose([1, 0, 2]))
                else:  # replicate bottom row (maxpool -inf pad)
                    nc.sync.dma_start(out=U[base + KB - 1:base + KB],
                                      in_=X[base + KB - 1:base + KB])
                # D[base+i] = x row q0+i-1
                nc.sync.dma_start(out=D[base + 1:base + KB],
                                  in_=X[base:base + KB - 1])
                if q0 > 0:
                    nc.gpsimd.dma_start(
                        out=D[base:base + 1],
                        in_=src[:, q0 - 1:q0, :].transpose([1, 0, 2]))
                else:  # replicate top row
                    nc.sync.dma_start(out=D[base:base + 1], in_=X[base:base + 1])

            # ---- pools: mx = 3x3 max (U/X/D vertical, slices horizontal);
            #             bh = horizontal box sum of X (zero pad).
            # mx/bh have a zero col on each side (data col j at tile col j+1)
            # so each kw tap is a full-width rhs slice with built-in zero pad.
            b_ = tmp_pool.tile([128, T, W], BF16, tag="poolb")
            vx = tmp_pool.tile([128, T, W], BF16, tag="poolvx")
            mx = mxbh_pool.tile([128, T, W + 2], BF16, tag="mx")
            bh = mxbh_pool.tile([128, T, W + 2], BF16, tag="bh")
            p = P3
            nc.vector.tensor_tensor(out=b_[:p], in0=U[:p], in1=D[:p], op=MAXOP)
            nc.vector.tensor_tensor(out=vx[:p], in0=b_[:p], in1=X[:p], op=MAXOP)
            for pad in (mx, bh):
                nc.vector.memset(pad[:p, :, 0:1], 0)
                nc.vector.memset(pad[:p, :, W + 1:W + 2], 0)
            # ones-row at partition P3 of mx: carries the bias via the start matmul
            nc.sync.dma_start(out=mx[P3:P3 + 1], in_=ones[:])
            nc.vector.tensor_tensor(out=mx[:p, :, 1:256], in0=vx[:p, :, 0:255],
                                    in1=vx[:p, :, 1:256], op=MAXOP)
            nc.vector.tensor_copy(mx[:p, :, 256:257], vx[:p, :, 255:256])
            nc.vector.tensor_tensor(out=mx[:p, :, 2:257], in0=mx[:p, :, 2:257],
                                    in1=vx[:p, :, 0:255], op=MAXOP)
            # horizontal box sum on GpSimd to unload DVE
            nc.gpsimd.tensor_tensor(out=bh[:p, :, 1:256], in0=X[:p, :, 0:255],
                                    in1=X[:p, :, 1:256], op=ADDOP)
            nc.gpsimd.tensor_copy(bh[:p, :, 256:257], X[:p, :, 255:256])
            nc.gpsimd.tensor_tensor(out=bh[:p, :, 2:257], in0=bh[:p, :, 2:257],
                                    in1=X[:p, :, 0:255], op=ADDOP)

            # ---- matmuls: operator-outer; all 16 ts resident in 8 PSUM banks
            ps = [psum_pool.tile([128, 2, W], F32, tag=f"ps{b}", name=f"ps{b}")
                  for b in range(8)]
            ops = [(path, kw) for path in (0, 1) for kw in (1, 0, 2)]
            n = len(ops)
            for i, (path, kw) in enumerate(ops):
                src = mx if path == 0 else bh
                mat = _mat_index(ch, path, kw)
                K = P3 + 1 if i == 0 else P3  # start matmul includes bias row
                lhsT = cst[0:K, mat, 0:M]
                for b in range(8):
                    rhs = src[0:K, 2 * b:2 * b + 2, kw:kw + W]
                    nc.tensor.matmul(ps[b][0:M], lhsT, rhs,
                                     start=(i == 0), stop=(i == n - 1))

            # ---- epilogue per bank: leaky (from PSUM) -> sigmoid -> 6 DMAs
            for b in range(8):
                lk = epi_pool.tile([128, 2, W], F32, tag="epil")
                nc.vector.tensor_scalar_mul(lk[0:M], ps[b][0:M], 0.01)
                lk2 = epi_pool.tile([128, 2, W], F32, tag="epil2")
                nc.vector.tensor_tensor(out=lk2[0:M], in0=lk[0:M], in1=ps[b][0:M],
                                        op=MAXOP)
                sg = epi_pool.tile([128, 2, W], F32, tag="epis")
                nc.scalar.activation(sg[0:M], lk2[0:M],
                                     mybir.ActivationFunctionType.Sigmoid)
                for tt in range(2):
                    t = 2 * b + tt
                    for c in range(3):
                        eng = (nc.sync, nc.scalar, nc.gpsimd)[c]
                        eng.dma_start(out=out_ap[3 * t + c, m0:m1, :],
                                      in_=sg[0:M, tt, :])

    nc.compile()
    return nc


def kernel(input_tensor, conv_w, conv_b):
    input_tensor = np.ascontiguousarray(np.asarray(input_tensor, dtype=np.float32))
    conv_w = np.asarray(conv_w, dtype=np.float32)
    conv_b = np.asarray(conv_b, dtype=np.float32)

    if "nc" not in _cache:
        _cache["nc"] = _build_program()
    nc = _cache["nc"]

    stack = _build_stack(conv_w, conv_b)
    in_maps = [
        {"x": input_tensor[i], "cst": stack}
        for i in range(N_CORES)
    ]
    res = run_bass_kernel_spmd(nc, in_maps, list(range(N_CORES)))
    out = np.stack([res.results[i]["out"] for i in range(N_CORES)], axis=0)
    return out.astype(np.float32)


if __name__ == "__main__":
    rng = np.random.default_rng(0)
    x = rng.standard_normal((B, CTOT, H, W), dtype=np.float32)
    cw = rng.uniform(-0.1, 0.1, (1, 6, 3, 3)).astype(np.float32)
    cb = np.array([0.01], dtype=np.float32)
    o = kernel(x, cw, cb)
    print(o.shape, o.dtype)


# revision 27
# speedup vs baseline: 2.3525x; 2.0859x over previous
"""Trainium2 Bass kernel for nn_CBAMSpaceMask (CBAM spatial mask over T timestep blocks).

Math per timestep block t (3 channels):
  mx_c = maxpool3x3(x_c)          (stride 1, -inf pad == replicate pad)
  av_c = avgpool3x3(x_c)/9        (zero pad, count_include_pad)
  y_t  = sum_c wM_c * mx_c + wA_c * av_c + b   (3x3 conv, zero pad)
  out[3t+c] = sigmoid(leakyrelu(y_t))          (broadcast over c)

Device decomposition (per core = 1 batch image):
  - interleaved partition layout p = 3*row + c; tiles [126, 16 ts, cols] bf16.
    X loaded once per chunk; U/D row shifts are whole-tile SBUF->SBUF DMA
    shifts by 3 partitions, 1-row patches from neighbor chunks' X tiles.
  - max path: 3x3 maxpool on DVE (vertical via U/X/D, horizontal via col
    slices) into mx with 1 zero pad col each side.
  - avg path: fully folded into the operator: horizontal box -> 5 kw taps on
    raw X (2 zero pad cols each side); vertical box + conv -> band-5 Toeplitz.
  - conv: channel-folded Toeplitz matmuls on PE (bf16): K = 126 (+bias row),
    M = y rows, N = 2048 (8 ts packed); two PSUM half-tiles ping-pong so the
    epilogue drain of one half overlaps matmuls of the other.
  - epilogue: sigmoid(leaky(v)) = max(sigmoid(0.01v), sigmoid(v)): two ACT
    sigmoids straight from PSUM (bf16 out) + one DVE bf16 max; 3 cast stores
    per (chunk, half) via the channel-sliced DRAM view.
Sharding: pure data parallel, batch dim across 8 cores.
"""
import sys

sys.path.insert(0, "/opt/trn_rl_repo")

import numpy as np
import ml_dtypes
from contextlib import ExitStack

import concourse.bass as bass
import concourse.tile as tile
from concourse import bacc, mybir
from concourse.bass_utils import run_bass_kernel_spmd

F32 = mybir.dt.float32
BF16 = mybir.dt.bfloat16

B, CTOT, H, W = 8, 48, 256, 256
T = 16
N_CORES = 8

# chunk geometry: y rows [m0, m1) from pooled rows [q0, q1); KB = q1 - q0
# q0 = max(0, m0-2) covers the band-5 avg operator; KB <= 42 so 3*KB <= 126.
CHUNKS = []
_m0 = 0
while _m0 < H:
    _m1 = min(_m0 + 38, H)
    _q0 = max(0, _m0 - 2)
    _q1 = min(_q0 + 42, H)
    CHUNKS.append((_m0, _m1, _q0, _q1))
    _m0 = _m1
NCH = len(CHUNKS)
NOPS = 8           # 3 max-path kw taps + 5 avg-path kw taps
NMAT = NCH * NOPS

_cache = {}


def _mat_index(ch, op):
    return ch * NOPS + op


def _build_stack(conv_w, conv_b):
    """lhsT stack [128, NMAT, 128] bf16, K interleaved as k = 3*i + c.

    Per chunk, 8 operators:
      op 0..2 (max path, kw): band-3 vertical conv applied to mx
      op 3..7 (avg path, j):  band-5 (conv @ Bv)/9 with horizontally
        box-composed kw weights, applied to raw X
    The bias rides row 3*KB of operator 0 (the start matmul).
    """
    w = conv_w[0].astype(np.float64)  # [6, 3, 3]
    Bv = np.zeros((H, H))
    for i in (-1, 0, 1):
        Bv += np.eye(H, k=i)
    stack = np.zeros((128, NMAT, 128), dtype=np.float64)
    for ch, (m0, m1, q0, q1) in enumerate(CHUNKS):
        M = m1 - m0
        KB = q1 - q0
        for c in range(3):
            kmax = w[2 * c]
            kavg = w[2 * c + 1]
            for kw in range(3):
                op = np.zeros((H, H))
                for kh in range(3):
                    op += kmax[kh, kw] * np.eye(H, k=kh - 1)
                stack[c * KB:(c + 1) * KB, _mat_index(ch, kw), 0:M] = op[m0:m1, q0:q1].T
            for j in range(5):
                s = j - 2
                op = np.zeros((H, H))
                for kh in range(3):
                    kc = sum(kavg[kh, kw] for kw in range(3) if abs(s - (kw - 1)) <= 1)
                    op += kc * np.eye(H, k=kh - 1)
                op = (op @ Bv) / 9.0
                stack[c * KB:(c + 1) * KB, _mat_index(ch, 3 + j), 0:M] = op[m0:m1, q0:q1].T
        stack[3 * KB, _mat_index(ch, 0), 0:M] = float(conv_b[0])
    return stack.astype(ml_dtypes.bfloat16)


def _build_program():
    nc = bacc.Bacc("TRN2", target_bir_lowering=False, debug=False, enable_asserts=False)
    x_ap = nc.dram_tensor("x", [CTOT, H, W], F32, kind="ExternalInput").ap()
    cst_ap = nc.dram_tensor("cst", [128, NMAT, 128], BF16, kind="ExternalInput").ap()
    out_ap = nc.dram_tensor("out", [CTOT, H, W], F32, kind="ExternalOutput").ap()
    # [16, 3, H, W] channel-sliced view of the output for per-channel stores
    out_il = out_ap.rearrange("(t c) h w -> t c h w", c=3)

    MAXOP = mybir.AluOpType.max
    SIG = mybir.ActivationFunctionType.Sigmoid

    with tile.TileContext(nc) as tc, ExitStack() as ctx:
        const_pool = ctx.enter_context(tc.tile_pool(name="const", bufs=1))
        x_pool = ctx.enter_context(tc.tile_pool(name="xload", bufs=1))
        ud_pool = ctx.enter_context(tc.tile_pool(name="udtiles", bufs=2))
        tmp_pool = ctx.enter_context(tc.tile_pool(name="ptmp", bufs=1))
        mx_pool = ctx.enter_context(tc.tile_pool(name="mxp", bufs=2))
        psum_pool = ctx.enter_context(tc.tile_pool(name="psum", bufs=1, space="PSUM"))
        epi_pool = ctx.enter_context(tc.tile_pool(name="epi", bufs=2))

        cst = const_pool.tile([128, NMAT, 128], BF16, tag="cst")
        nc.sync.dma_start(out=cst[:], in_=cst_ap)
        ones = const_pool.tile([1, T, W + 4], BF16, tag="ones")
        nc.vector.memset(ones[:], 1.0)

        # ---- preload all chunks' X tiles: data cols at [2, W+2), 2 zero pad
        # cols each side for the avg path's 5-tap horizontal reads
        Xs = []
        for ch, (m0, m1, q0, q1) in enumerate(CHUNKS):
            KB = q1 - q0
            X = x_pool.tile([128, T, W + 4], BF16, tag=f"ldx{ch}", name=f"ldx{ch}")
            for c in range(3):
                nc.gpsimd.dma_start(
                    out=X[c * KB:(c + 1) * KB, :, 2:W + 2],
                    in_=x_ap[c::3][:, q0:q1, :].transpose([1, 0, 2]))
            nc.vector.memset(X[0:3 * KB, :, 0:2], 0)
            nc.vector.memset(X[0:3 * KB, :, W + 2:W + 4], 0)
            Xs.append(X)

        for ch, (m0, m1, q0, q1) in enumerate(CHUNKS):
            M = m1 - m0
            KB = q1 - q0
            P3 = 3 * KB
            X = Xs[ch]

            # ---- U/D: per-channel-block partition shifts + 1-row patches
            U = ud_pool.tile([128, T, W + 4], BF16, tag="ldu")
            D = ud_pool.tile([128, T, W + 4], BF16, tag="ldd")
            for c in range(3):
                cb = c * KB
                nc.gpsimd.dma_start(out=U[cb:cb + KB - 1], in_=X[cb + 1:cb + KB])
                if q1 < H:  # row q1 lives in the next chunk's X
                    _, _, q0n, q1n = CHUNKS[ch + 1]
                    kbn = q1n - q0n
                    i = c * kbn + (q1 - q0n)
                    nc.sync.dma_start(out=U[cb + KB - 1:cb + KB],
                                      in_=Xs[ch + 1][i:i + 1])
                else:  # replicate bottom row (maxpool -inf pad)
                    nc.sync.dma_start(out=U[cb + KB - 1:cb + KB],
                                      in_=X[cb + KB - 1:cb + KB])
                nc.gpsimd.dma_start(out=D[cb + 1:cb + KB], in_=X[cb:cb + KB - 1])
                if q0 > 0:  # row q0-1 lives in the previous chunk's X
                    _, _, q0p, q1p = CHUNKS[ch - 1]
                    kbp = q1p - q0p
                    i = c * kbp + (q0 - 1 - q0p)
                    nc.sync.dma_start(out=D[cb:cb + 1], in_=Xs[ch - 1][i:i + 1])
                else:  # replicate top row
                    nc.sync.dma_start(out=D[cb:cb + 1], in_=X[cb:cb + 1])

            # ---- max pool on DVE; mx data cols at [1, W+1), zero pad sides
            b_ = tmp_pool.tile([128, T, W + 4], BF16, tag="poolb")
            vx = tmp_pool.tile([128, T, W + 4], BF16, tag="poolvx")
            mx = mx_pool.tile([128, T, W + 2], BF16, tag="mx")
            p = P3
            nc.vector.tensor_tensor(out=b_[:p, :, 2:W + 2], in0=U[:p, :, 2:W + 2],
                                    in1=D[:p, :, 2:W + 2], op=MAXOP)
            nc.vector.tensor_tensor(out=vx[:p, :, 2:W + 2], in0=b_[:p, :, 2:W + 2],
                                    in1=X[:p, :, 2:W + 2], op=MAXOP)
            nc.vector.memset(mx[:p, :, 0:1], 0)
            nc.vector.memset(mx[:p, :, W + 1:W + 2], 0)
            # ones-row at partition P3 of mx: carries the bias via the start matmul
            nc.sync.dma_start(out=mx[P3:P3 + 1], in_=ones[:, :, 0:W + 2])
            # data col w at mx col w+1; vx data col w at vx col w+2
            nc.vector.tensor_tensor(out=mx[:p, :, 1:256], in0=vx[:p, :, 2:257],
                                    in1=vx[:p, :, 3:258], op=MAXOP)
            nc.vector.tensor_copy(mx[:p, :, 256:257], vx[:p, :, 257:258])
            nc.vector.tensor_tensor(out=mx[:p, :, 2:257], in0=mx[:p, :, 2:257],
                                    in1=vx[:p, :, 2:257], op=MAXOP)

            # ---- matmuls: operator-outer; all 16 ts resident in 8 PSUM banks
            ps = [psum_pool.tile([128, 2, W], F32, tag=f"ps{b}", name=f"ps{b}")
                  for b in range(8)]
            for i in range(NOPS):
                K = P3 + 1 if i == 0 else P3  # start matmul adds bias row
                lhsT = cst[0:K, _mat_index(ch, i), 0:M]
                src, off = (mx, i) if i < 3 else (X, i - 3)
                for b in range(8):
                    rhs = src[0:K, 2 * b:2 * b + 2, off:off + W]
                    nc.tensor.matmul(ps[b][0:M], lhsT, rhs,
                                     start=(i == 0), stop=(i == NOPS - 1))

            # ---- epilogue: max(sigmoid(0.01v), sigmoid(v)) == sigmoid(leaky(v))
            sg = epi_pool.tile([128, T, W], BF16, tag="episg")
            for b in range(8):
                s1 = epi_pool.tile([128, 2, W], BF16, tag="epis1")
                nc.scalar.activation(s1[0:M], ps[b][0:M], SIG, scale=0.01)
                s2 = epi_pool.tile([128, 2, W], BF16, tag="epis2")
                nc.scalar.activation(s2[0:M], ps[b][0:M], SIG)
                nc.vector.tensor_tensor(out=sg[0:M, 2 * b:2 * b + 2, :],
                                        in0=s1[0:M], in1=s2[0:M], op=MAXOP)
            for c in range(3):
                dst = out_il[:, c, m0:m1, :].transpose([1, 0, 2])
                nc.gpsimd.dma_start(out=dst, in_=sg[0:M])

    nc.compile()
    return nc


def kernel(input_tensor, conv_w, conv_b):
    input_tensor = np.ascontiguousarray(np.asarray(input_tensor, dtype=np.float32))
    conv_w = np.asarray(conv_w, dtype=np.float32)
    conv_b = np.asarray(conv_b, dtype=np.float32)

    if "nc" not in _cache:
        _cache["nc"] = _build_program()
    nc = _cache["nc"]

    stack = _build_stack(conv_w, conv_b)
    in_maps = [
        {"x": input_tensor[i], "cst": stack}
        for i in range(N_CORES)
    ]
    res = run_bass_kernel_spmd(nc, in_maps, list(range(N_CORES)))
    out = np.stack([res.results[i]["out"] for i in range(N_CORES)], axis=0)
    return out.astype(np.float32)


if __name__ == "__main__":
    rng = np.random.default_rng(0)
    x = rng.standard_normal((B, CTOT, H, W), dtype=np.float32)
    cw = rng.uniform(-0.1, 0.1, (1, 6, 3, 3)).astype(np.float32)
    cb = np.array([0.01], dtype=np.float32)
    o = kernel(x, cw, cb)
    print(o.shape, o.dtype)
